# revision 9
# baseline (speedup 1.0000x reference)
"""Trainium2 Bass kernel for CapsuleLayer dynamic routing.

Problem: inputs [64, 2048, 8] f32, W [32, 2048, 16, 8] f32
  inputs_hat[b,n,i,e] = sum_d inputs[b,i,d] * W[n,i,e,d]
  3 routing iterations (softmax over n, weighted sums over i, squash)
  -> outputs [64, 32, 16] f32

Strategy: data-parallel over batch across 8 cores (8 batches each, W
replicated).  Per core:
  Phase 1 (memory-bound): stream W (pre-cast bf16 on host, pre-tiled
    into 128 chunks of [128=(i16,d8), 512=(n,e)]) and compute
    inputs_hat via block-diagonal-inputs matmuls on the PE; evacuate
    PSUM->SBUF bf16 (ACT/DVE alternating); simultaneously accumulate
    the uniform-c s_0 = sum_i inputs_hat via a fixed delta-mask matmul.
  Phase 2 (routing tail, on-chip): layout ih[p=(i16,b8), f=(e,k,n)].
    b-update: DVE mul (2x bf16) + contiguous e-halving tree (2x).
    softmax over n: ACT exp + DVE reduce/reciprocal.
    s_r = sum_i c*ih: DVE mul then fixed delta-mask PE matmuls
    accumulating all 128 chunks into one PSUM bank (avoids 1x
    tensor_reduce).  squash via Ln/Exp (one ACT table set).
"""

import numpy as np

B, I, DI = 64, 2048, 8
N, DO = 32, 16
CORES = 8
BL = B // CORES  # 8 batches per core
KC = 128         # i-chunks
ISUB = 16        # i per chunk
FNE = N * DO     # 512
KB = 8           # chunks per tail block
NBLK = KC // KB  # 16 tail blocks
EPS = 1e-7

_CACHE = {}


def _patch_tile_tail_barrier():
    """The walrus build in this container rejects >limit sync-waits on the
    Tile tail Drain. Replace the multi-wait drain with one wait_ge per
    outstanding semaphore (SP executes them in order), then a bare drain."""
    import concourse.tile as tile

    if getattr(tile.TileContext, "_ant_split_drain_patch", False):
        return

    def _drain_and_barrier(self, tick_clock, wait_clock):
        gc = tick_clock.global_clock
        ticks = eval(repr(gc).replace("VectorClock(", "").rstrip(")"))
        for idx, sem in sorted(self.sems.allocated().items()):
            if idx < len(ticks) and ticks[idx] > 0:
                mult = 16 if idx >= 11 else 1
                self.nc.sync.wait_ge(sem, ticks[idx] * mult)
        self.nc.sync.drain()
        self.nc.all_engine_barrier()
        popped = self.nc._tile_sem_poison_stack.pop()
        assert popped is self._sem_poison
        self.nc.clear_and_free_semaphores(list(self.sems.allocated().values()))

    tile.TileContext._drain_and_barrier = _drain_and_barrier
    tile.TileContext._ant_split_drain_patch = True


def _split_multi_waits(bir_bytes):
    """This container's walrus build allows only one sync-wait per
    instruction.  Hoist extra semaphore waits onto preceding wait-only
    EventSemaphore instructions on the same engine (engines execute their
    stream in order, so semantics are preserved)."""
    import json

    d = json.loads(bir_bytes)
    ctr = 0
    for f in d["functions"]:
        for blk in f["blocks"]:
            out = []
            for ins in blk["instructions"]:
                waits = ins.get("sync_info", {}).get("on_wait", [])
                if len(waits) > 1:
                    for w in waits[:-1]:
                        ctr += 1
                        out.append({
                            "debug": ins.get("debug", 0),
                            "engine": ins["engine"],
                            "ins": [],
                            "name": f"antwaitsplit-{ctr}",
                            "opcode": "EventSemaphore",
                            "outs": [],
                            "sync_info": {"on_update": [], "on_wait": [w]},
                        })
                    ins["sync_info"]["on_wait"] = [waits[-1]]
                out.append(ins)
            blk["instructions"] = out
    return json.dumps(d).encode()


def _patch_compile_split_waits():
    from concourse import bass2jax, bass_utils

    if getattr(bass_utils, "_ant_split_waits_patch", False):
        return
    orig = bass_utils.compile_bir_kernel

    def patched(bir_json, tmpdir, neff_name="file.neff"):
        return orig(_split_multi_waits(bir_json), tmpdir, neff_name)

    bass_utils.compile_bir_kernel = patched
    bass_utils._ant_split_waits_patch = True
    if getattr(bass2jax, "compile_bir_kernel", None) is orig:
        bass2jax.compile_bir_kernel = patched


def _build_nc():
    import concourse.bass as bass
    import concourse.tile as tile
    from concourse import mybir

    _patch_tile_tail_barrier()
    _patch_compile_split_waits()

    f32 = mybir.dt.float32
    bf16 = mybir.dt.bfloat16
    AF = mybir.ActivationFunctionType
    OP = mybir.AluOpType
    AX = mybir.AxisListType

    nc = bass.Bass(target_bir_lowering=False)

    wprep = nc.dram_tensor("wprep", [KC, 128, FNE], bf16, kind="ExternalInput")
    binp = nc.dram_tensor("binp", [KC, 128, 128], bf16, kind="ExternalInput")
    dmask = nc.dram_tensor("dmask", [128, BL], bf16, kind="ExternalInput")
    out_d = nc.dram_tensor("out", [BL, FNE], f32, kind="ExternalOutput")

    with tile.TileContext(nc) as tc:
        with (
            tc.tile_pool(name="big", bufs=1) as big,
            tc.tile_pool(name="wstream", bufs=4) as wpool,
            tc.tile_pool(name="bstream", bufs=4) as bpool,
            tc.tile_pool(name="mmpsum", bufs=4, space="PSUM") as mmp,
            tc.tile_pool(name="spsum", bufs=2, space="PSUM") as spp,
            tc.tile_pool(name="blk", bufs=2) as blkpool,
            tc.tile_pool(name="small", bufs=1) as small,
            tc.tile_pool(name="consts", bufs=1) as consts,
        ):
            # persistent tensors
            ih = big.tile([128, DO, KC, N], bf16, name="ih")  # [p,(e,k,n)]
            b_acc = big.tile([128, KB, NBLK, N], bf16, name="b_acc")
            dm = consts.tile([128, BL], bf16, name="dm")
            nc.sync.dma_start(dm[:], dmask[:])
            epsb = consts.tile([BL, 1], f32, name="epsb")
            nc.vector.memset(epsb[:], EPS)

            # ---------------- Phase 1: W stream ----------------
            s0 = spp.tile([BL, FNE], f32, name="s0")
            for k in range(KC):
                wc = wpool.tile([128, FNE], bf16, name="wc")
                nc.sync.dma_start(wc[:], wprep[k])
                bp = bpool.tile([128, 128], bf16, name="bp")
                nc.sync.dma_start(bp[:], binp[k])
                ps = mmp.tile([128, FNE], f32, name="ps")
                nc.tensor.matmul(ps[:], bp[:], wc[:], start=True, stop=True,
                                 skip_group_check=True)
                # evacuate PSUM [(i,b),(n,e)] -> ih[p, e, k, n] bf16
                dst = ih[:, :, k, :]
                src = ps[:].rearrange("p (n e) -> p e n", e=DO)
                if k % 2 == 0:
                    nc.scalar.copy(dst, src)
                else:
                    nc.vector.tensor_copy(dst, src)
                # s0 accumulation: sum_i ih (delta-mask matmul)
                rhs = ih[:, :, k, :].rearrange("p e n -> p n e")
                nc.tensor.matmul(s0[:], dm[:], rhs, start=(k == 0),
                                 stop=(k == KC - 1), skip_group_check=True)

            # ---------------- squash helper ----------------
            def squash(s_psum, r):
                s_sb = small.tile([BL, FNE], f32, name="s_sb", tag="s_sb")
                scale0 = (1.0 / N) if r == 0 else 1.0
                nc.scalar.mul(s_sb[:], s_psum[:], scale0)
                sqv = small.tile([BL, FNE], f32, name="sqv", tag="sqv")
                nc.vector.tensor_mul(sqv[:], s_sb[:], s_sb[:])
                s2 = small.tile([BL, N], f32, name="s2", tag="s2")
                nc.vector.tensor_reduce(
                    s2[:], sqv[:].rearrange("b (n e) -> b n e", e=DO),
                    axis=AX.X, op=OP.add)
                l1 = small.tile([BL, N], f32, name="l1", tag="l1")
                nc.scalar.activation(l1[:], s2[:], AF.Ln, bias=1.0)
                l2 = small.tile([BL, N], f32, name="l2", tag="l2")
                nc.scalar.activation(l2[:], s2[:], AF.Ln, bias=epsb[:])
                tt = small.tile([BL, N], f32, name="tt", tag="tt")
                nc.vector.tensor_scalar_mul(tt[:], l2[:], -0.5)
                nc.vector.tensor_sub(tt[:], tt[:], l1[:])
                sc = small.tile([BL, N], f32, name="sc", tag="sc")
                nc.scalar.activation(sc[:], tt[:], AF.Exp)
                nc.vector.tensor_mul(sc[:], sc[:], s2[:])
                v_f = small.tile([BL, N, DO], f32, name="v_f", tag="v_f")
                nc.vector.tensor_tensor(
                    v_f[:], s_sb[:].rearrange("b (n e) -> b n e", e=DO),
                    sc[:, :, None].broadcast_to([BL, N, DO]), op=OP.mult)
                return v_f

            v_f = squash(s0, 0)

            # ---------------- routing iterations ----------------
            for r in (1, 2):
                # replicate v into [p=(i16,b8), e, n]
                v_bf = small.tile([BL, DO, N], bf16, name="v_bf", tag="v_bf")
                nc.vector.tensor_copy(
                    v_bf[:], v_f[:].rearrange("b n e -> b e n"))
                vrep = small.tile([128, DO, N], bf16, name="vrep", tag="vrep")
                for g in range(ISUB):
                    nc.sync.dma_start(
                        vrep[g * BL:(g + 1) * BL, :, :], v_bf[:])

                s_ps = spp.tile([BL, FNE], f32, name="s_ps", tag="s_ps")
                for blk in range(NBLK):
                    ihb = ih[:, :, blk * KB:(blk + 1) * KB, :]  # [p,e,kb,n]
                    # --- b-update: p2 = ih * v, tree-reduce over e ---
                    p2 = blkpool.tile([128, DO, KB, N], bf16, name="p2",
                                      tag="p2")
                    nc.vector.tensor_tensor(
                        p2[:], ihb,
                        vrep[:, :, None, :].broadcast_to([128, DO, KB, N]),
                        op=OP.mult)
                    h = DO
                    while h > 2:
                        h //= 2
                        nc.vector.tensor_add(
                            p2[:, 0:h], p2[:, 0:h], p2[:, h:2 * h])
                    bslc = b_acc[:, :, blk, :]
                    if r == 1:
                        nc.vector.tensor_add(bslc, p2[:, 0], p2[:, 1])
                    else:
                        nc.vector.tensor_add(p2[:, 0], p2[:, 0], p2[:, 1])
                        nc.vector.tensor_add(bslc, bslc, p2[:, 0])
                    # --- softmax over n ---
                    eb = blkpool.tile([128, KB, N], bf16, name="eb", tag="eb")
                    nc.scalar.activation(eb[:], bslc, AF.Exp)
                    ns = blkpool.tile([128, KB], f32, name="ns", tag="ns")
                    nc.vector.tensor_reduce(ns[:], eb[:], axis=AX.X, op=OP.add)
                    rec = blkpool.tile([128, KB], f32, name="rec", tag="rec")
                    nc.vector.reciprocal(rec[:], ns[:])
                    cb = blkpool.tile([128, KB, N], bf16, name="cb", tag="cb")
                    cslc = cb[:]
                    nc.vector.tensor_tensor(
                        cslc, eb[:],
                        rec[:, :, None].broadcast_to([128, KB, N]),
                        op=OP.mult)
                    # --- s partial: p3 = ih * c, PE-accumulate over i ---
                    p3 = blkpool.tile([128, DO, KB, N], bf16, name="p3",
                                      tag="p3")
                    nc.vector.tensor_tensor(
                        p3[:], ihb,
                        cslc[:, None, :, :].broadcast_to([128, DO, KB, N]),
                        op=OP.mult)
                    for kk in range(KB):
                        k = blk * KB + kk
                        nc.tensor.matmul(
                            s_ps[:], dm[:],
                            p3[:, :, kk, :].rearrange("p e n -> p n e"),
                            start=(k == 0), stop=(k == KC - 1),
                            skip_group_check=True)

                v_f = squash(s_ps, r)

            nc.sync.dma_start(out_d[:], v_f[:].rearrange("b n e -> b (n e)"))

    return nc


def _host_prep(inputs, W):
    import ml_dtypes
    bf = ml_dtypes.bfloat16

    # W_prep [128, 128, 512]: [k, (i16,d8), (n,e)]
    wt = np.transpose(W, (1, 3, 0, 2))  # [i, d, n, e]
    wprep = np.ascontiguousarray(
        wt.reshape(KC, ISUB * DI, N * DO)).astype(bf)

    # delta mask [128=(i16,b8), 8]
    dmask = np.tile(np.eye(BL, dtype=np.float32), (ISUB, 1)).astype(bf)

    in_maps = []
    for c in range(CORES):
        ic = inputs[c * BL:(c + 1) * BL]  # [8, 2048, 8]
        base = np.transpose(ic, (1, 2, 0)).reshape(KC, ISUB, DI, BL)
        bd = np.einsum("kidb,ij->kidjb", base.astype(np.float32),
                       np.eye(ISUB, dtype=np.float32))
        binp = np.ascontiguousarray(
            bd.reshape(KC, ISUB * DI, ISUB * BL)).astype(bf)
        in_maps.append({"wprep": wprep, "binp": binp, "dmask": dmask})
    return in_maps


def kernel(inputs, W):
    from concourse.bass_utils import run_bass_kernel_spmd

    inputs = np.asarray(inputs, dtype=np.float32)
    W = np.asarray(W, dtype=np.float32)

    if "nc" not in _CACHE:
        _CACHE["nc"] = _build_nc()
    nc = _CACHE["nc"]

    in_maps = _host_prep(inputs, W)
    res = run_bass_kernel_spmd(nc, in_maps, core_ids=list(range(CORES)))
    outs = [res.results[c]["out"].reshape(BL, N, DO) for c in range(CORES)]
    return np.concatenate(outs, axis=0).astype(np.float32)


# revision 11
# speedup vs baseline: 1.0822x; 1.0822x over previous
"""Trainium2 Bass kernel for CapsuleLayer dynamic routing.

Problem: inputs [64, 2048, 8] f32, W [32, 2048, 16, 8] f32
  inputs_hat[b,n,i,e] = sum_d inputs[b,i,d] * W[n,i,e,d]
  3 routing iterations (softmax over n, weighted sums over i, squash)
  -> outputs [64, 32, 16] f32

Strategy: data-parallel over batch across 8 cores (8 batches each, W
replicated).  Per core:
  Phase 1: stream W (pre-cast bf16, pre-tiled on host into 128 chunks of
    [128=(i16,d8), 512=(e,n)]) and compute inputs_hat via
    block-diagonal-inputs matmuls on the PE; evacuate PSUM->SBUF bf16;
    simultaneously accumulate the uniform-c s_0 = sum_i inputs_hat via a
    fixed delta-mask matmul (contiguous rhs).
  Phase 2 (routing tail, on-chip): ih[p=(i16,b8), f=(k,e,n)] so the
    weighted i-reduction s_r = sum_i c*ih runs as contiguous-rhs
    delta-mask PE matmuls accumulating into one PSUM bank, and every big
    DVE multiply hits the 2x bf16 mode (innermost n, step 1; broadcasts
    on outer/middle dims).  b-update via DVE mul + contiguous e-halving
    tree.  softmax over n on ACT exp + DVE reduce/reciprocal.  squash via
    Ln/Exp (one ACT table set, no sqrt table switch).
"""

import numpy as np

B, I, DI = 64, 2048, 8
N, DO = 32, 16
CORES = 8
BL = B // CORES  # 8 batches per core
KC = 128         # i-chunks
ISUB = 16        # i per chunk
FNE = N * DO     # 512
KB = 8           # chunks per tail block
NBLK = KC // KB  # 16 tail blocks
EPS = 1e-7
SPLIT_GPS = False  # split big DVE ops with GPSIMD

_CACHE = {}


def _patch_tile_tail_barrier():
    """The walrus build in this container rejects >1 sync-wait on the Tile
    tail Drain.  Replace the multi-wait drain with one wait_ge per
    outstanding semaphore (SP executes them in order), then a bare drain."""
    import concourse.tile as tile

    if getattr(tile.TileContext, "_ant_split_drain_patch", False):
        return

    def _drain_and_barrier(self, tick_clock, wait_clock):
        gc = tick_clock.global_clock
        ticks = eval(repr(gc).replace("VectorClock(", "").rstrip(")"))
        for idx, sem in sorted(self.sems.allocated().items()):
            if idx < len(ticks) and ticks[idx] > 0:
                mult = 16 if idx >= 11 else 1
                self.nc.sync.wait_ge(sem, ticks[idx] * mult)
        self.nc.sync.drain()
        self.nc.all_engine_barrier()
        popped = self.nc._tile_sem_poison_stack.pop()
        assert popped is self._sem_poison
        self.nc.clear_and_free_semaphores(list(self.sems.allocated().values()))

    tile.TileContext._drain_and_barrier = _drain_and_barrier
    tile.TileContext._ant_split_drain_patch = True


def _split_multi_waits(bir_bytes):
    """This container's walrus build allows only one sync-wait per
    instruction.  Hoist extra semaphore waits onto preceding wait-only
    EventSemaphore instructions on the same engine (engines execute their
    stream in order, so semantics are preserved)."""
    import json

    d = json.loads(bir_bytes)
    ctr = 0
    for f in d["functions"]:
        for blk in f["blocks"]:
            out = []
            for ins in blk["instructions"]:
                waits = ins.get("sync_info", {}).get("on_wait", [])
                if len(waits) > 1:
                    for w in waits[:-1]:
                        ctr += 1
                        out.append({
                            "debug": ins.get("debug", 0),
                            "engine": ins["engine"],
                            "ins": [],
                            "name": f"antwaitsplit-{ctr}",
                            "opcode": "EventSemaphore",
                            "outs": [],
                            "sync_info": {"on_update": [], "on_wait": [w]},
                        })
                    ins["sync_info"]["on_wait"] = [waits[-1]]
                out.append(ins)
            blk["instructions"] = out
    return json.dumps(d).encode()


def _patch_compile_split_waits():
    from concourse import bass2jax, bass_utils

    if getattr(bass_utils, "_ant_split_waits_patch", False):
        return
    orig = bass_utils.compile_bir_kernel

    def patched(bir_json, tmpdir, neff_name="file.neff"):
        return orig(_split_multi_waits(bir_json), tmpdir, neff_name)

    bass_utils.compile_bir_kernel = patched
    bass_utils._ant_split_waits_patch = True
    if getattr(bass2jax, "compile_bir_kernel", None) is orig:
        bass2jax.compile_bir_kernel = patched


def _build_nc():
    import concourse.bass as bass
    import concourse.tile as tile
    from concourse import mybir

    _patch_tile_tail_barrier()
    _patch_compile_split_waits()

    f32 = mybir.dt.float32
    bf16 = mybir.dt.bfloat16
    AF = mybir.ActivationFunctionType
    OP = mybir.AluOpType
    AX = mybir.AxisListType

    nc = bass.Bass(target_bir_lowering=False)

    wprep = nc.dram_tensor("wprep", [KC, 128, FNE], bf16, kind="ExternalInput")
    binp = nc.dram_tensor("binp", [KC, 128, 128], bf16, kind="ExternalInput")
    dmask = nc.dram_tensor("dmask", [128, BL], bf16, kind="ExternalInput")
    out_d = nc.dram_tensor("out", [BL, FNE], f32, kind="ExternalOutput")

    dma_engines = [nc.sync]

    with tile.TileContext(nc) as tc:
        with (
            tc.tile_pool(name="big", bufs=1) as big,
            tc.tile_pool(name="wstream", bufs=6) as wpool,
            tc.tile_pool(name="bstream", bufs=6) as bpool,
            tc.tile_pool(name="mmpsum", bufs=4, space="PSUM") as mmp,
            tc.tile_pool(name="spsum", bufs=2, space="PSUM") as spp,
            tc.tile_pool(name="blk", bufs=2) as blkpool,
            tc.tile_pool(name="small", bufs=1) as small,
            tc.tile_pool(name="consts", bufs=1) as consts,
        ):
            # persistent tensors; ih free dims = (k, e, n)
            ih = big.tile([128, KC, DO, N], bf16, name="ih")
            b_acc = big.tile([128, KC, N], bf16, name="b_acc")
            dm = consts.tile([128, BL], bf16, name="dm")
            nc.sync.dma_start(dm[:], dmask[:])
            epsb = consts.tile([BL, 1], f32, name="epsb")
            nc.vector.memset(epsb[:], EPS)

            # ---------------- Phase 1: W stream ----------------
            s0 = spp.tile([BL, FNE], f32, name="s0")
            for k in range(KC):
                eng = dma_engines[k % len(dma_engines)]
                wc = wpool.tile([128, FNE], bf16, name="wc")
                eng.dma_start(wc[:], wprep[k])
                bp = bpool.tile([128, 128], bf16, name="bp")
                eng.dma_start(bp[:], binp[k])
                ps = mmp.tile([128, FNE], f32, name="ps")
                nc.tensor.matmul(ps[:], bp[:], wc[:], start=True, stop=True,
                                 skip_group_check=True)
                # evacuate PSUM [(i,b),(e,n)] -> ih[p, k, e, n] bf16
                dst = ih[:, k, :, :].rearrange("p e n -> p (e n)")
                if k % 2 == 0:
                    nc.scalar.copy(dst, ps[:])
                else:
                    nc.vector.tensor_copy(dst, ps[:])
                # s0 accumulation: sum_i ih (delta-mask matmul, contiguous)
                nc.tensor.matmul(s0[:], dm[:],
                                 ih[:, k, :, :].rearrange("p e n -> p (e n)"),
                                 start=(k == 0), stop=(k == KC - 1),
                                 skip_group_check=True)

            # ---------------- squash helper (cols are (e, n)) ---------
            def squash(s_psum, r):
                s_sb = small.tile([BL, FNE], f32, name="s_sb", tag="s_sb")
                scale0 = (1.0 / N) if r == 0 else 1.0
                nc.scalar.mul(s_sb[:], s_psum[:], scale0)
                sqv = small.tile([BL, FNE], f32, name="sqv", tag="sqv")
                nc.vector.tensor_mul(sqv[:], s_sb[:], s_sb[:])
                s2 = small.tile([BL, N], f32, name="s2", tag="s2")
                nc.vector.tensor_reduce(
                    s2[:], sqv[:].rearrange("b (e n) -> b n e", e=DO),
                    axis=AX.X, op=OP.add)
                l1 = small.tile([BL, N], f32, name="l1", tag="l1")
                nc.scalar.activation(l1[:], s2[:], AF.Ln, bias=1.0)
                l2 = small.tile([BL, N], f32, name="l2", tag="l2")
                nc.scalar.activation(l2[:], s2[:], AF.Ln, bias=epsb[:])
                tt = small.tile([BL, N], f32, name="tt", tag="tt")
                nc.vector.tensor_scalar_mul(tt[:], l2[:], -0.5)
                nc.vector.tensor_sub(tt[:], tt[:], l1[:])
                sc = small.tile([BL, N], f32, name="sc", tag="sc")
                nc.scalar.activation(sc[:], tt[:], AF.Exp)
                nc.vector.tensor_mul(sc[:], sc[:], s2[:])
                v_f = small.tile([BL, DO, N], f32, name="v_f", tag="v_f")
                nc.vector.tensor_tensor(
                    v_f[:], s_sb[:].rearrange("b (e n) -> b e n", e=DO),
                    sc[:, None, :].broadcast_to([BL, DO, N]), op=OP.mult)
                return v_f

            v_f = squash(s0, 0)

            # ---------------- routing iterations ----------------
            for r in (1, 2):
                v_bf = small.tile([BL, DO, N], bf16, name="v_bf", tag="v_bf")
                nc.vector.tensor_copy(v_bf[:], v_f[:])
                vrep = small.tile([128, DO, N], bf16, name="vrep", tag="vrep")
                for g in range(ISUB):
                    dma_engines[g % len(dma_engines)].dma_start(
                        vrep[g * BL:(g + 1) * BL, :, :], v_bf[:])

                s_ps = spp.tile([BL, FNE], f32, name="s_ps", tag="s_ps")
                for blk in range(NBLK):
                    ihb = ih[:, blk * KB:(blk + 1) * KB, :, :]  # [p,kb,e,n]
                    # --- b-update: p2 = ih * v, tree-reduce over e ---
                    p2 = blkpool.tile([128, KB, DO, N], bf16, name="p2",
                                      tag="p2")
                    vb = vrep[:, None, :, :].broadcast_to([128, KB, DO, N])
                    if SPLIT_GPS:
                        h = KB // 2
                        nc.vector.tensor_tensor(
                            p2[:, :h], ihb[:, :h], vb[:, :h], op=OP.mult)
                        nc.gpsimd.tensor_tensor(
                            p2[:, h:], ihb[:, h:], vb[:, h:], op=OP.mult)
                    else:
                        nc.vector.tensor_tensor(p2[:], ihb, vb, op=OP.mult)
                    h = DO
                    while h > 2:
                        h //= 2
                        if SPLIT_GPS:
                            m = KB // 2
                            nc.vector.tensor_add(
                                p2[:, :m, 0:h], p2[:, :m, 0:h],
                                p2[:, :m, h:2 * h])
                            nc.gpsimd.tensor_add(
                                p2[:, m:, 0:h], p2[:, m:, 0:h],
                                p2[:, m:, h:2 * h])
                        else:
                            nc.vector.tensor_add(
                                p2[:, :, 0:h], p2[:, :, 0:h], p2[:, :, h:2 * h])
                    bslc = b_acc[:, blk * KB:(blk + 1) * KB, :]
                    if r == 1:
                        nc.vector.tensor_add(bslc, p2[:, :, 0, :],
                                             p2[:, :, 1, :])
                    else:
                        nc.vector.tensor_add(p2[:, :, 0, :], p2[:, :, 0, :],
                                             p2[:, :, 1, :])
                        nc.vector.tensor_add(bslc, bslc, p2[:, :, 0, :])
                    # --- softmax over n ---
                    eb = blkpool.tile([128, KB, N], bf16, name="eb", tag="eb")
                    nc.scalar.activation(eb[:], bslc, AF.Exp)
                    ns = blkpool.tile([128, KB], f32, name="ns", tag="ns")
                    nc.vector.tensor_reduce(ns[:], eb[:], axis=AX.X, op=OP.add)
                    rec = blkpool.tile([128, KB], f32, name="rec", tag="rec")
                    nc.vector.reciprocal(rec[:], ns[:])
                    cb = blkpool.tile([128, KB, N], bf16, name="cb", tag="cb")
                    nc.vector.tensor_tensor(
                        cb[:], eb[:],
                        rec[:, :, None].broadcast_to([128, KB, N]),
                        op=OP.mult)
                    # --- s partial: p3 = ih * c, PE-accumulate over i ---
                    p3 = blkpool.tile([128, KB, DO, N], bf16, name="p3",
                                      tag="p3")
                    cbb = cb[:, :, None, :].broadcast_to([128, KB, DO, N])
                    if SPLIT_GPS:
                        h = KB // 2
                        nc.vector.tensor_tensor(
                            p3[:, :h], ihb[:, :h], cbb[:, :h], op=OP.mult)
                        nc.gpsimd.tensor_tensor(
                            p3[:, h:], ihb[:, h:], cbb[:, h:], op=OP.mult)
                    else:
                        nc.vector.tensor_tensor(p3[:], ihb, cbb, op=OP.mult)
                    for kk in range(KB):
                        k = blk * KB + kk
                        nc.tensor.matmul(
                            s_ps[:], dm[:],
                            p3[:, kk, :, :].rearrange("p e n -> p (e n)"),
                            start=(k == 0), stop=(k == KC - 1),
                            skip_group_check=True)

                v_f = squash(s_ps, r)

            nc.sync.dma_start(out_d[:], v_f[:].rearrange("b e n -> b (e n)"))

    return nc


def _host_prep(inputs, W):
    import ml_dtypes
    bf = ml_dtypes.bfloat16

    # W_prep [128, 128, 512]: [k, (i16,d8), (e,n)]
    wt = np.transpose(W, (1, 3, 2, 0))  # [i, d, e, n]
    wprep = np.ascontiguousarray(
        wt.reshape(KC, ISUB * DI, DO * N)).astype(bf)

    # delta mask [128=(i16,b8), 8]
    dmask = np.tile(np.eye(BL, dtype=np.float32), (ISUB, 1)).astype(bf)

    in_maps = []
    for c in range(CORES):
        ic = inputs[c * BL:(c + 1) * BL]  # [8, 2048, 8]
        base = np.transpose(ic, (1, 2, 0)).reshape(KC, ISUB, DI, BL)
        bd = np.einsum("kidb,ij->kidjb", base.astype(np.float32),
                       np.eye(ISUB, dtype=np.float32))
        binp = np.ascontiguousarray(
            bd.reshape(KC, ISUB * DI, ISUB * BL)).astype(bf)
        in_maps.append({"wprep": wprep, "binp": binp, "dmask": dmask})
    return in_maps


def kernel(inputs, W):
    from concourse.bass_utils import run_bass_kernel_spmd

    inputs = np.asarray(inputs, dtype=np.float32)
    W = np.asarray(W, dtype=np.float32)

    if "nc" not in _CACHE:
        _CACHE["nc"] = _build_nc()
    nc = _CACHE["nc"]

    in_maps = _host_prep(inputs, W)
    res = run_bass_kernel_spmd(nc, in_maps, core_ids=list(range(CORES)))
    outs = [res.results[c]["out"].reshape(BL, DO, N).transpose(0, 2, 1)
            for c in range(CORES)]
    return np.concatenate(outs, axis=0).astype(np.float32)


# revision 12
# speedup vs baseline: 1.2113x; 1.1193x over previous
"""Trainium2 Bass kernel for CapsuleLayer dynamic routing.

Problem: inputs [64, 2048, 8] f32, W [32, 2048, 16, 8] f32
  inputs_hat[b,n,i,e] = sum_d inputs[b,i,d] * W[n,i,e,d]
  3 routing iterations (softmax over n, weighted sums over i, squash)
  -> outputs [64, 32, 16] f32

Strategy: data-parallel over batch across 8 cores (8 batches each, W
replicated).  Per core:
  Phase 1: stream W (pre-cast bf16, pre-tiled on host into 128 chunks of
    [128=(i16,d8), 512=(e,n)]) and compute inputs_hat via
    block-diagonal-inputs matmuls on the PE; evacuate PSUM->SBUF bf16;
    simultaneously accumulate the uniform-c s_0 = sum_i inputs_hat via a
    fixed delta-mask matmul (contiguous rhs).
  Phase 2 (routing tail, on-chip): ih[p=(i16,b8), f=(k,e,n)] so the
    weighted i-reduction s_r = sum_i c*ih runs as contiguous-rhs
    delta-mask PE matmuls accumulating into one PSUM bank, and every big
    DVE multiply hits the 2x bf16 mode (innermost n, step 1; broadcasts
    on outer/middle dims).  b-update via DVE mul + contiguous e-halving
    tree.  softmax over n on ACT exp + DVE reduce/reciprocal.  squash via
    Ln/Exp (one ACT table set, no sqrt table switch).
"""

import numpy as np

B, I, DI = 64, 2048, 8
N, DO = 32, 16
CORES = 8
BL = B // CORES  # 8 batches per core
KC = 128         # i-chunks
ISUB = 16        # i per chunk
FNE = N * DO     # 512
KB = 8           # chunks per tail block
NBLK = KC // KB  # 16 tail blocks
EPS = 1e-7
SPLIT_GPS = False  # split big DVE ops with GPSIMD

_CACHE = {}


def _patch_tile_tail_barrier():
    """The walrus build in this container rejects >1 sync-wait on the Tile
    tail Drain.  Replace the multi-wait drain with one wait_ge per
    outstanding semaphore (SP executes them in order), then a bare drain."""
    import concourse.tile as tile

    if getattr(tile.TileContext, "_ant_split_drain_patch", False):
        return

    def _drain_and_barrier(self, tick_clock, wait_clock):
        gc = tick_clock.global_clock
        ticks = eval(repr(gc).replace("VectorClock(", "").rstrip(")"))
        for idx, sem in sorted(self.sems.allocated().items()):
            if idx < len(ticks) and ticks[idx] > 0:
                mult = 16 if idx >= 11 else 1
                self.nc.sync.wait_ge(sem, ticks[idx] * mult)
        self.nc.sync.drain()
        self.nc.all_engine_barrier()
        popped = self.nc._tile_sem_poison_stack.pop()
        assert popped is self._sem_poison
        self.nc.clear_and_free_semaphores(list(self.sems.allocated().values()))

    tile.TileContext._drain_and_barrier = _drain_and_barrier
    tile.TileContext._ant_split_drain_patch = True


def _split_multi_waits(bir_bytes):
    """This container's walrus build allows only one sync-wait per
    instruction.  Hoist extra semaphore waits onto preceding wait-only
    EventSemaphore instructions on the same engine (engines execute their
    stream in order, so semantics are preserved)."""
    import json

    d = json.loads(bir_bytes)
    ctr = 0
    for f in d["functions"]:
        for blk in f["blocks"]:
            out = []
            for ins in blk["instructions"]:
                waits = ins.get("sync_info", {}).get("on_wait", [])
                if len(waits) > 1:
                    for w in waits[:-1]:
                        ctr += 1
                        out.append({
                            "debug": ins.get("debug", 0),
                            "engine": ins["engine"],
                            "ins": [],
                            "name": f"antwaitsplit-{ctr}",
                            "opcode": "EventSemaphore",
                            "outs": [],
                            "sync_info": {"on_update": [], "on_wait": [w]},
                        })
                    ins["sync_info"]["on_wait"] = [waits[-1]]
                out.append(ins)
            blk["instructions"] = out
    return json.dumps(d).encode()


def _patch_compile_split_waits():
    from concourse import bass2jax, bass_utils

    if getattr(bass_utils, "_ant_split_waits_patch", False):
        return
    orig = bass_utils.compile_bir_kernel

    def patched(bir_json, tmpdir, neff_name="file.neff"):
        return orig(_split_multi_waits(bir_json), tmpdir, neff_name)

    bass_utils.compile_bir_kernel = patched
    bass_utils._ant_split_waits_patch = True
    if getattr(bass2jax, "compile_bir_kernel", None) is orig:
        bass2jax.compile_bir_kernel = patched


def _build_nc():
    import concourse.bass as bass
    import concourse.tile as tile
    from concourse import mybir

    _patch_tile_tail_barrier()
    _patch_compile_split_waits()

    f32 = mybir.dt.float32
    bf16 = mybir.dt.bfloat16
    AF = mybir.ActivationFunctionType
    OP = mybir.AluOpType
    AX = mybir.AxisListType

    nc = bass.Bass(target_bir_lowering=False)

    wprep = nc.dram_tensor("wprep", [KC, 128, FNE], bf16, kind="ExternalInput")
    binp = nc.dram_tensor("binp", [KC, 128, 128], bf16, kind="ExternalInput")
    dmask = nc.dram_tensor("dmask", [128, BL], bf16, kind="ExternalInput")
    out_d = nc.dram_tensor("out", [BL, FNE], f32, kind="ExternalOutput")

    dma_engines = [nc.sync, nc.gpsimd]

    with tile.TileContext(nc) as tc:
        with (
            tc.tile_pool(name="big", bufs=1) as big,
            tc.tile_pool(name="wstream", bufs=6) as wpool,
            tc.tile_pool(name="bstream", bufs=6) as bpool,
            tc.tile_pool(name="mmpsum", bufs=4, space="PSUM") as mmp,
            tc.tile_pool(name="spsum", bufs=2, space="PSUM") as spp,
            tc.tile_pool(name="blk", bufs=2) as blkpool,
            tc.tile_pool(name="small", bufs=1) as small,
            tc.tile_pool(name="consts", bufs=1) as consts,
        ):
            # persistent tensors; ih free dims = (k, e, n)
            ih = big.tile([128, KC, DO, N], bf16, name="ih")
            b_acc = big.tile([128, KC, N], bf16, name="b_acc")
            dm = consts.tile([128, BL], bf16, name="dm")
            nc.sync.dma_start(dm[:], dmask[:])
            epsb = consts.tile([BL, 1], f32, name="epsb")
            nc.vector.memset(epsb[:], EPS)

            # ---------------- Phase 1: W stream ----------------
            s0 = spp.tile([BL, FNE], f32, name="s0")
            for k in range(KC):
                eng = dma_engines[k % len(dma_engines)]
                wc = wpool.tile([128, FNE], bf16, name="wc")
                eng.dma_start(wc[:], wprep[k])
                bp = bpool.tile([128, 128], bf16, name="bp")
                eng.dma_start(bp[:], binp[k])
                ps = mmp.tile([128, FNE], f32, name="ps")
                nc.tensor.matmul(ps[:], bp[:], wc[:], start=True, stop=True,
                                 skip_group_check=True)
                # evacuate PSUM [(i,b),(e,n)] -> ih[p, k, e, n] bf16
                dst = ih[:, k, :, :].rearrange("p e n -> p (e n)")
                if k % 2 == 0:
                    nc.scalar.copy(dst, ps[:])
                else:
                    nc.vector.tensor_copy(dst, ps[:])
                # s0 accumulation: sum_i ih (delta-mask matmul, contiguous)
                nc.tensor.matmul(s0[:], dm[:],
                                 ih[:, k, :, :].rearrange("p e n -> p (e n)"),
                                 start=(k == 0), stop=(k == KC - 1),
                                 skip_group_check=True)

            # ---------------- squash helper (cols are (e, n)) ---------
            def squash(s_psum, r):
                s_sb = small.tile([BL, FNE], f32, name="s_sb", tag="s_sb")
                scale0 = (1.0 / N) if r == 0 else 1.0
                nc.scalar.mul(s_sb[:], s_psum[:], scale0)
                sqv = small.tile([BL, FNE], f32, name="sqv", tag="sqv")
                nc.vector.tensor_mul(sqv[:], s_sb[:], s_sb[:])
                s2 = small.tile([BL, N], f32, name="s2", tag="s2")
                nc.vector.tensor_reduce(
                    s2[:], sqv[:].rearrange("b (e n) -> b n e", e=DO),
                    axis=AX.X, op=OP.add)
                l1 = small.tile([BL, N], f32, name="l1", tag="l1")
                nc.scalar.activation(l1[:], s2[:], AF.Ln, bias=1.0)
                l2 = small.tile([BL, N], f32, name="l2", tag="l2")
                nc.scalar.activation(l2[:], s2[:], AF.Ln, bias=epsb[:])
                tt = small.tile([BL, N], f32, name="tt", tag="tt")
                nc.vector.tensor_scalar_mul(tt[:], l2[:], -0.5)
                nc.vector.tensor_sub(tt[:], tt[:], l1[:])
                sc = small.tile([BL, N], f32, name="sc", tag="sc")
                nc.scalar.activation(sc[:], tt[:], AF.Exp)
                nc.vector.tensor_mul(sc[:], sc[:], s2[:])
                v_f = small.tile([BL, DO, N], f32, name="v_f", tag="v_f")
                nc.vector.tensor_tensor(
                    v_f[:], s_sb[:].rearrange("b (e n) -> b e n", e=DO),
                    sc[:, None, :].broadcast_to([BL, DO, N]), op=OP.mult)
                return v_f

            v_f = squash(s0, 0)

            # ---------------- routing iterations ----------------
            for r in (1, 2):
                v_bf = small.tile([BL, DO, N], bf16, name="v_bf", tag="v_bf")
                nc.vector.tensor_copy(v_bf[:], v_f[:])
                vrep = small.tile([128, DO, N], bf16, name="vrep", tag="vrep")
                for g in range(ISUB):
                    dma_engines[g % len(dma_engines)].dma_start(
                        vrep[g * BL:(g + 1) * BL, :, :], v_bf[:])

                s_ps = spp.tile([BL, FNE], f32, name="s_ps", tag="s_ps")
                for blk in range(NBLK):
                    ihb = ih[:, blk * KB:(blk + 1) * KB, :, :]  # [p,kb,e,n]
                    # --- b-update: p2 = ih * v, tree-reduce over e ---
                    p2 = blkpool.tile([128, KB, DO, N], bf16, name="p2",
                                      tag="p2")
                    vb = vrep[:, None, :, :].broadcast_to([128, KB, DO, N])
                    if SPLIT_GPS:
                        h = KB // 2
                        nc.vector.tensor_tensor(
                            p2[:, :h], ihb[:, :h], vb[:, :h], op=OP.mult)
                        nc.gpsimd.tensor_tensor(
                            p2[:, h:], ihb[:, h:], vb[:, h:], op=OP.mult)
                    else:
                        nc.vector.tensor_tensor(p2[:], ihb, vb, op=OP.mult)
                    h = DO
                    while h > 2:
                        h //= 2
                        if SPLIT_GPS:
                            m = KB // 2
                            nc.vector.tensor_add(
                                p2[:, :m, 0:h], p2[:, :m, 0:h],
                                p2[:, :m, h:2 * h])
                            nc.gpsimd.tensor_add(
                                p2[:, m:, 0:h], p2[:, m:, 0:h],
                                p2[:, m:, h:2 * h])
                        else:
                            nc.vector.tensor_add(
                                p2[:, :, 0:h], p2[:, :, 0:h], p2[:, :, h:2 * h])
                    bslc = b_acc[:, blk * KB:(blk + 1) * KB, :]
                    if r == 1:
                        nc.vector.tensor_add(bslc, p2[:, :, 0, :],
                                             p2[:, :, 1, :])
                    else:
                        nc.vector.tensor_add(p2[:, :, 0, :], p2[:, :, 0, :],
                                             p2[:, :, 1, :])
                        nc.vector.tensor_add(bslc, bslc, p2[:, :, 0, :])
                    # --- softmax over n ---
                    eb = blkpool.tile([128, KB, N], bf16, name="eb", tag="eb")
                    nc.scalar.activation(eb[:], bslc, AF.Exp)
                    ns = blkpool.tile([128, KB], f32, name="ns", tag="ns")
                    nc.vector.tensor_reduce(ns[:], eb[:], axis=AX.X, op=OP.add)
                    rec = blkpool.tile([128, KB], f32, name="rec", tag="rec")
                    nc.vector.reciprocal(rec[:], ns[:])
                    cb = blkpool.tile([128, KB, N], bf16, name="cb", tag="cb")
                    nc.vector.tensor_tensor(
                        cb[:], eb[:],
                        rec[:, :, None].broadcast_to([128, KB, N]),
                        op=OP.mult)
                    # --- s partial: p3 = ih * c, PE-accumulate over i ---
                    p3 = blkpool.tile([128, KB, DO, N], bf16, name="p3",
                                      tag="p3")
                    cbb = cb[:, :, None, :].broadcast_to([128, KB, DO, N])
                    if SPLIT_GPS:
                        h = KB // 2
                        nc.vector.tensor_tensor(
                            p3[:, :h], ihb[:, :h], cbb[:, :h], op=OP.mult)
                        nc.gpsimd.tensor_tensor(
                            p3[:, h:], ihb[:, h:], cbb[:, h:], op=OP.mult)
                    else:
                        nc.vector.tensor_tensor(p3[:], ihb, cbb, op=OP.mult)
                    for kk in range(KB):
                        k = blk * KB + kk
                        nc.tensor.matmul(
                            s_ps[:], dm[:],
                            p3[:, kk, :, :].rearrange("p e n -> p (e n)"),
                            start=(k == 0), stop=(k == KC - 1),
                            skip_group_check=True)

                v_f = squash(s_ps, r)

            nc.sync.dma_start(out_d[:], v_f[:].rearrange("b e n -> b (e n)"))

    return nc


def _host_prep(inputs, W):
    import ml_dtypes
    bf = ml_dtypes.bfloat16

    # W_prep [128, 128, 512]: [k, (i16,d8), (e,n)]
    wt = np.transpose(W, (1, 3, 2, 0))  # [i, d, e, n]
    wprep = np.ascontiguousarray(
        wt.reshape(KC, ISUB * DI, DO * N)).astype(bf)

    # delta mask [128=(i16,b8), 8]
    dmask = np.tile(np.eye(BL, dtype=np.float32), (ISUB, 1)).astype(bf)

    in_maps = []
    for c in range(CORES):
        ic = inputs[c * BL:(c + 1) * BL]  # [8, 2048, 8]
        base = np.transpose(ic, (1, 2, 0)).reshape(KC, ISUB, DI, BL)
        bd = np.einsum("kidb,ij->kidjb", base.astype(np.float32),
                       np.eye(ISUB, dtype=np.float32))
        binp = np.ascontiguousarray(
            bd.reshape(KC, ISUB * DI, ISUB * BL)).astype(bf)
        in_maps.append({"wprep": wprep, "binp": binp, "dmask": dmask})
    return in_maps


def kernel(inputs, W):
    from concourse.bass_utils import run_bass_kernel_spmd

    inputs = np.asarray(inputs, dtype=np.float32)
    W = np.asarray(W, dtype=np.float32)

    if "nc" not in _CACHE:
        _CACHE["nc"] = _build_nc()
    nc = _CACHE["nc"]

    in_maps = _host_prep(inputs, W)
    res = run_bass_kernel_spmd(nc, in_maps, core_ids=list(range(CORES)))
    outs = [res.results[c]["out"].reshape(BL, DO, N).transpose(0, 2, 1)
            for c in range(CORES)]
    return np.concatenate(outs, axis=0).astype(np.float32)
